# revision 24
# baseline (speedup 1.0000x reference)
"""Trainium2 Bass kernel for a 6-layer post-LN Transformer encoder.

Strategy (8 NeuronCores):
  - Sequence-parallel: cores 0-3 own batch 0, cores 4-7 own batch 1; each core
    owns 512 tokens. Weights are replicated (bf16).
  - Attention is computed flash-style against the LOCAL key/value shard only
    (512 keys), producing an unnormalized attention output plus the softmax
    denominator (via a ones-column appended to V). The partials are summed
    over the 4-core group with bf16 AllReduces; normalization happens after.
  - The attention runs as two query-half sweeps; the first AllReduce is
    issued mid-attention so it overlaps the second sweep, and the layer tail
    (normalize -> Wo -> LN1 -> FFN -> LN2) is software-pipelined over the two
    query/token halves so LayerNorm vector/scalar chains overlap TensorE work.
  - Activations are kept feature-major ([D, tok]) on-chip; LayerNorm
    statistics use ones-vector matmuls (partition-direction reduction).
"""

import numpy as np
import ml_dtypes

L, D, H, FF = 6, 1024, 16, 4096
DK = D // H          # 64
B, S = 2, 2048
NCORES = 8
R = 4                # cores per batch group
T = S // R           # 512 local tokens per core
HT = T // 2          # 256 token/query half
DC = D // 128        # 8
FC = FF // 128       # 32
TC = T // 128        # 4  (local key chunks)
EPS = 1e-5
BF16 = ml_dtypes.bfloat16

_CACHE = {}


def _build_nc():
    import contextlib
    import concourse.bacc as bacc
    import concourse.mybir as mybir
    import concourse.tile as tile
    import concourse.bass as bass
    from concourse.bass import ts, ds

    f32 = mybir.dt.float32
    bf16 = mybir.dt.bfloat16
    AF = mybir.ActivationFunctionType
    OP = mybir.AluOpType

    nc = bacc.Bacc(num_devices=NCORES)

    # ---- parameters -----------------------------------------------------
    x0T = nc.declare_dram_parameter("x0T", [D, T], f32, isOutput=False)
    maskb = nc.declare_dram_parameter("maskb", [128, TC], f32, isOutput=False)
    wq = nc.declare_dram_parameter("wq", [L, D, D], bf16, isOutput=False)
    wk = nc.declare_dram_parameter("wk", [L, D, D], bf16, isOutput=False)
    wv = nc.declare_dram_parameter("wv", [L, D, D], bf16, isOutput=False)
    wo = nc.declare_dram_parameter("wo", [L, D, D], bf16, isOutput=False)
    w1 = nc.declare_dram_parameter("w1", [L, D, FF], bf16, isOutput=False)
    # W2 pre-packed on host: [L, mc(8), o(32), p(128), m(128)]
    w2p = nc.declare_dram_parameter("w2p", [L, DC, FC, 128, 128], bf16, isOutput=False)
    bq = nc.declare_dram_parameter("bq", [L, D], f32, isOutput=False)
    bk = nc.declare_dram_parameter("bk", [L, D], f32, isOutput=False)
    bvb = nc.declare_dram_parameter("bvb", [L, D], bf16, isOutput=False)
    bo = nc.declare_dram_parameter("bo", [L, D], f32, isOutput=False)
    b1 = nc.declare_dram_parameter("b1", [L, FF], f32, isOutput=False)
    b2 = nc.declare_dram_parameter("b2", [L, D], f32, isOutput=False)
    g1 = nc.declare_dram_parameter("g1", [L, D], f32, isOutput=False)
    be1 = nc.declare_dram_parameter("be1", [L, D], f32, isOutput=False)
    g2 = nc.declare_dram_parameter("g2", [L, D], f32, isOutput=False)
    be2 = nc.declare_dram_parameter("be2", [L, D], f32, isOutput=False)
    outT = nc.declare_dram_parameter("outT", [D, T], f32, isOutput=True)

    groups = [[0, 1, 2, 3], [4, 5, 6, 7]]

    with tile.TileContext(nc) as tc:
        ctx = contextlib.ExitStack()
        singles = ctx.enter_context(tc.tile_pool(name="singles", bufs=1))
        params = ctx.enter_context(tc.tile_pool(name="params", bufs=2))
        wpool = ctx.enter_context(tc.tile_pool(name="wpool", bufs=2))
        w2pool = ctx.enter_context(tc.tile_pool(name="w2pool", bufs=2))
        ktpool = ctx.enter_context(tc.tile_pool(name="ktpool", bufs=2))
        qtpool = ctx.enter_context(tc.tile_pool(name="qtpool", bufs=2))
        vpool = ctx.enter_context(tc.tile_pool(name="vpool", bufs=2))
        apool = ctx.enter_context(tc.tile_pool(name="apool", bufs=8))
        xbfpool = ctx.enter_context(tc.tile_pool(name="xbfpool", bufs=2))
        anfpool = ctx.enter_context(tc.tile_pool(name="anfpool", bufs=4))
        ao65pool = ctx.enter_context(tc.tile_pool(name="ao65pool", bufs=4))
        tmp = ctx.enter_context(tc.tile_pool(name="tmp", bufs=2))
        small = ctx.enter_context(tc.tile_pool(name="small", bufs=2))
        denp = ctx.enter_context(tc.tile_pool(name="denp", bufs=2))
        rbp = ctx.enter_context(tc.tile_pool(name="rbp", bufs=2))
        dram = ctx.enter_context(tc.tile_pool(name="dram", bufs=2, space="DRAM"))
        pscore = ctx.enter_context(tc.tile_pool(name="pscore", bufs=2, space="PSUM"))
        pav = ctx.enter_context(tc.tile_pool(name="pav", bufs=2, space="PSUM"))
        pmisc = ctx.enter_context(tc.tile_pool(name="pmisc", bufs=2, space="PSUM"))

        # ---- constants + resident state --------------------------------
        xT = singles.tile([128, DC, T], f32, name="xT")
        nc.sync.dma_start(out=xT, in_=x0T[:, :].rearrange("(c p) t -> p c t", p=128))
        mb_sb = singles.tile([128, TC], f32, name="mb_sb")
        nc.sync.dma_start(out=mb_sb, in_=maskb[:, :])
        ones_col = singles.tile([128, 1], f32, name="ones_col")
        nc.vector.memset(ones_col, 1.0)
        ones_row = singles.tile([1, 128], f32, name="ones_row")
        nc.vector.memset(ones_row, 1.0)
        ones_row_bf = singles.tile([1, 128], bf16, name="ones_row_bf")
        nc.vector.memset(ones_row_bf, 1.0)
        eps_sb = singles.tile([1, 1], f32, name="eps_sb")
        nc.vector.memset(eps_sb, EPS)
        xbf_cur = xbfpool.tile([128, DC, T], bf16, tag="xbf")
        for c in range(DC):
            nc.vector.tensor_copy(xbf_cur[:, c, :], xT[:, c, :])

        def layernorm_half(g_sb, be_sb, hq, xbf_out, late=False):
            """LN of token half hq: x = LN(x) in place + bf16 copy to xbf_out.

            late=True routes the bf16 cast to GpSimd — only safe when the
            LN is ordered after this layer's AllReduce waits on that queue.
            """
            q0 = hq * HT
            psum_sum = pmisc.tile([128, 512], f32, tag="pmisc")
            for c in range(DC):
                nc.tensor.matmul(psum_sum[0:1, 0:HT], ones_col,
                                 xT[:, c, ds(q0, HT)],
                                 start=(c == 0), stop=(c == DC - 1))
            psum_sq = pmisc.tile([128, 512], f32, tag="pmisc")
            for c in range(DC):
                sq = tmp.tile([128, HT], f32, tag="tmp")
                nc.vector.tensor_mul(sq, xT[:, c, ds(q0, HT)], xT[:, c, ds(q0, HT)])
                nc.tensor.matmul(psum_sq[0:1, 0:HT], ones_col, sq,
                                 start=(c == 0), stop=(c == DC - 1))
            mr = small.tile([1, 512], f32, tag="mr")
            e2 = small.tile([1, HT], f32, tag="e2")
            msq = small.tile([1, HT], f32, tag="msq")
            nc.scalar.mul(mr[:, 0:HT], psum_sum[0:1, 0:HT], 1.0 / D)
            nc.scalar.mul(e2, psum_sq[0:1, 0:HT], 1.0 / D)
            nc.vector.tensor_mul(msq, mr[:, 0:HT], mr[:, 0:HT])
            nc.vector.tensor_tensor(e2, e2, msq, OP.subtract)
            lnv = small.tile([1, HT], f32, tag="lnv")
            nc.scalar.activation(lnv, e2, AF.Ln, bias=eps_sb)
            nc.scalar.activation(mr[:, 256:256 + HT], lnv, AF.Exp, scale=-0.5)
            # broadcast mean/rstd rows to all 128 partitions via a stride-0
            # DMA round trip (keeps PSUM free for matmuls)
            mrd = dram.tile([1, 512], f32, tag="mrd")
            nc.sync.dma_start(out=mrd[:, :], in_=mr)
            bcs = small.tile([128, 512], f32, tag="bcs")
            nc.sync.dma_start(
                out=bcs,
                in_=bass.AP(tensor=mrd.tensor, offset=mrd.offset,
                            ap=[[0, 128], [1, 512]]))
            bcm, bcr = bcs[:, 0:HT], bcs[:, 256:256 + HT]
            for c in range(DC):
                t1 = tmp.tile([128, HT], f32, tag="tmp")
                nc.vector.tensor_tensor(t1, xT[:, c, ds(q0, HT)], bcm,
                                        OP.subtract)
                nc.vector.tensor_tensor(t1, t1, bcr, OP.mult)
                nc.scalar.activation(xT[:, c, ds(q0, HT)], t1, AF.Identity,
                                     scale=g_sb[:, c:c + 1],
                                     bias=be_sb[:, c:c + 1])
                cast_eng = nc.gpsimd if late else nc.vector
                cast_eng.tensor_copy(xbf_out[:, c, ds(q0, HT)],
                                     xT[:, c, ds(q0, HT)])

        for l in range(L):
            # ---- per-layer params --------------------------------------
            pp = params.tile([128, 8, DC], f32, tag="pcol")
            for i_, t_src in enumerate([bq, bk, bo, b2, g1, be1, g2, be2]):
                nc.sync.dma_start(out=pp[:, i_, :],
                                  in_=t_src[l].rearrange("(c p) -> p c", p=128))
            bq_sb, bk_sb, bo_sb, b2_sb = pp[:, 0], pp[:, 1], pp[:, 2], pp[:, 3]
            g1_sb, be1_sb, g2_sb, be2_sb = pp[:, 4], pp[:, 5], pp[:, 6], pp[:, 7]
            b1_sb = params.tile([128, FC], f32, tag="pc32")
            nc.sync.dma_start(out=b1_sb, in_=b1[l].rearrange("(c p) -> p c", p=128))
            bv_row = params.tile([1, D], bf16, tag="bv_row")
            nc.sync.dma_start(out=bv_row, in_=bvb[l][None, :])

            xbf = xbf_cur

            # ---- K projection (local keys, feature-major) --------------
            wk_sb = wpool.tile([128, DC, D], bf16, tag="w")
            nc.scalar.dma_start(out=wk_sb, in_=wk[l].rearrange("(c p) m -> p c m", p=128))
            kt_sb = ktpool.tile([128, DC, T], bf16, tag="kt")
            for mc in range(DC):
                ps = pmisc.tile([128, 512], f32, tag="pmisc")
                for c in range(DC):
                    nc.tensor.matmul(ps, wk_sb[:, c, ts(mc, 128)], xbf[:, c, :],
                                     start=(c == 0), stop=(c == DC - 1))
                nc.scalar.activation(kt_sb[:, mc, :], ps, AF.Identity,
                                     bias=bk_sb[:, mc:mc + 1])

            # ---- Q projection ------------------------------------------
            wq_sb = wpool.tile([128, DC, D], bf16, tag="w")
            nc.scalar.dma_start(out=wq_sb, in_=wq[l].rearrange("(c p) m -> p c m", p=128))
            qT = qtpool.tile([128, DC, T], bf16, tag="qt")
            for mc in range(DC):
                ps = pmisc.tile([128, 512], f32, tag="pmisc")
                for c in range(DC):
                    nc.tensor.matmul(ps, wq_sb[:, c, ts(mc, 128)], xbf[:, c, :],
                                     start=(c == 0), stop=(c == DC - 1))
                nc.scalar.activation(qT[:, mc, :], ps, AF.Identity,
                                     bias=bq_sb[:, mc:mc + 1])

            # ---- V projection (token-major, 65-col per head) -----------
            wv_sb = wpool.tile([128, DC, D], bf16, tag="w")
            nc.scalar.dma_start(out=wv_sb, in_=wv[l].rearrange("(c p) m -> p c m", p=128))
            v65 = vpool.tile([128, TC, H * 65], bf16, tag="v65")
            for t_ in range(TC):
                v65v = v65[:, t_, :].rearrange("p (h w) -> p h w", w=65)
                nc.vector.memset(v65v[:, :, 64:65], 1.0)
                for nh in range(2):
                    ps = pmisc.tile([128, 512], f32, tag="pmisc")
                    for c in range(DC):
                        nc.tensor.matmul(ps, xbf[:, c, ts(t_, 128)],
                                         wv_sb[:, c, ds(nh * 512, 512)],
                                         start=(c == 0), stop=False)
                    nc.tensor.matmul(ps, ones_row_bf, bv_row[:, ds(nh * 512, 512)],
                                     start=False, stop=True)
                    nc.scalar.activation(
                        v65v[:, ds(nh * 8, 8), 0:64],
                        ps.rearrange("p (h d) -> p h d", d=64), AF.Copy)

            # ---- attention vs local KV shard, two query-half sweeps ----
            att_d = [dram.tile([16 * 65, HT], bf16, tag="attd0", name="attd0"),
                     dram.tile([16 * 65, HT], bf16, tag="attd1", name="attd1")]
            att_g = [dram.tile([16 * 65, HT], bf16, tag="attg0", name="attg0"),
                     dram.tile([16 * 65, HT], bf16, tag="attg1", name="attg1")]
            for hq in range(2):
                q0 = hq * HT
                for j in range(DC):  # head pairs (2j, 2j+1)
                    at_tiles = []
                    for kc in range(TC):
                        # the two row-tiled matmuls run concurrently on the PE
                        # -> their outputs must land in different PSUM banks
                        pss = pscore.tile([128, 1024], f32, tag="pscore")
                        nc.tensor.matmul(pss[:, 0:HT],
                                         kt_sb[0:64, j, ts(kc, 128)],
                                         qT[0:64, j, ds(q0, HT)],
                                         start=True, stop=True,
                                         tile_position=(0, 0))
                        nc.tensor.matmul(pss[:, 512:512 + HT],
                                         kt_sb[64:128, j, ts(kc, 128)],
                                         qT[64:128, j, ds(q0, HT)],
                                         start=True, stop=True,
                                         tile_position=(64, 0))
                        at = apool.tile([128, 512], bf16, tag="attn")
                        nc.scalar.activation(
                            at.rearrange("p (b x) -> p b x", b=2),
                            pss.rearrange("p (b x) -> p b x", b=2)[:, :, 0:HT],
                            AF.Exp, scale=1.0 / 32.0,
                            bias=mb_sb[:, kc:kc + 1])
                        at_tiles.append(at)
                    for ab in range(2):
                        h = 2 * j + ab
                        pav_t = pav.tile([65, 512], f32, tag="pav")
                        for kc in range(TC):
                            nc.tensor.matmul(pav_t[:, 0:HT],
                                             v65[:, kc, ds(h * 65, 65)],
                                             at_tiles[kc][:, ds(ab * HT, HT)],
                                             start=(kc == 0), stop=(kc == TC - 1))
                        ao65 = ao65pool.tile([65, HT], bf16, tag="ao65")
                        nc.vector.tensor_copy(ao65, pav_t[:, 0:HT])
                        nc.scalar.dma_start(out=att_d[hq][ds(h * 65, 65), :], in_=ao65)
                nc.gpsimd.collective_compute(
                    "AllReduce", OP.add, replica_groups=groups,
                    ins=[att_d[hq].opt()], outs=[att_g[hq].opt()])

            # ---- tail: normalize -> Wo -> LN1 -> (FFN, LN2 below) ------
            wo_sb = wpool.tile([128, DC, D], bf16, tag="w")
            nc.scalar.dma_start(out=wo_sb, in_=wo[l].rearrange("(c p) m -> p c m", p=128))
            an_bf = xbfpool.tile([128, DC, T], bf16, tag="xbf")
            den_sb = denp.tile([16, 512], bf16, tag="den")
            den_r = denp.tile([16, 512], bf16, tag="denr")
            denr_d = dram.tile([16, 512], bf16, tag="denr")
            xbf2 = xbfpool.tile([128, DC, T], bf16, tag="xbf")
            for hq in range(2):
                q0 = hq * HT
                nc.sync.dma_start(
                    out=den_sb[:, ds(q0, HT)],
                    in_=att_g[hq].rearrange("(h w) q -> h w q", w=65)[:, 64, :])
                with nc.allow_low_precision(reason="softmax denom recip bf16"):
                    nc.vector.reciprocal(den_r[:, ds(q0, HT)],
                                         den_sb[:, ds(q0, HT)])
                nc.sync.dma_start(out=denr_d[:, ds(q0, HT)],
                                    in_=den_r[:, ds(q0, HT)])
                for j in range(DC):
                    anf = anfpool.tile([128, HT], bf16, tag="anf")
                    for ab in range(2):
                        h = 2 * j + ab
                        nc.sync.dma_start(out=anf[ds(ab * 64, 64), :],
                                            in_=att_g[hq][ds(h * 65, 64), :])
                    rb = rbp.tile([128, HT], bf16, tag="rb")
                    for ab in range(2):
                        src = bass.AP(tensor=denr_d.tensor,
                                      offset=denr_d.offset + (2 * j + ab) * 512 + q0,
                                      ap=[[0, 64], [1, HT]])
                        nc.sync.dma_start(out=rb[ds(ab * 64, 64), :], in_=src)
                    nc.vector.tensor_tensor(an_bf[:, j, ds(q0, HT)], anf, rb,
                                            OP.mult)
                # Wo + residual for this half
                for mc in range(DC):
                    ps = pmisc.tile([128, 512], f32, tag="pmisc")
                    for c in range(DC):
                        nc.tensor.matmul(ps[:, 0:HT], wo_sb[:, c, ts(mc, 128)],
                                         an_bf[:, c, ds(q0, HT)],
                                         start=(c == 0), stop=(c == DC - 1))
                    nc.vector.scalar_tensor_tensor(
                        xT[:, mc, ds(q0, HT)], ps[:, 0:HT],
                        bo_sb[:, mc:mc + 1], xT[:, mc, ds(q0, HT)],
                        OP.add, OP.add)
                layernorm_half(g1_sb, be1_sb, hq, xbf2)

            # ---- FFN (weights outer, token halves inner) ---------------
            ffT = vpool.tile([128, FC, T], bf16, tag="fft", bufs=1)
            for b4 in range(4):
                w1b = wpool.tile([128, DC, 1024], bf16, tag="w")
                nc.scalar.dma_start(
                    out=w1b,
                    in_=w1[l][:, ds(b4 * 1024, 1024)].rearrange("(c p) m -> p c m", p=128))
                for hq in range(2):
                    q0 = hq * HT
                    for mcb in range(DC):
                        mc = b4 * 8 + mcb
                        ps = pmisc.tile([128, 512], f32, tag="pmisc")
                        for c in range(DC):
                            nc.tensor.matmul(ps[:, 0:HT], w1b[:, c, ts(mcb, 128)],
                                             xbf2[:, c, ds(q0, HT)],
                                             start=(c == 0), stop=(c == DC - 1))
                        nc.scalar.activation(ffT[:, mc, ds(q0, HT)], ps[:, 0:HT],
                                             AF.Relu, bias=b1_sb[:, mc:mc + 1])
            xbf_cur = xbfpool.tile([128, DC, T], bf16, tag="xbf")
            for mc in range(DC):
                w2mc = w2pool.tile([128, FC, 128], bf16, tag="w2")
                nc.scalar.dma_start(out=w2mc,
                                  in_=w2p[l, mc].rearrange("o p m -> p o m"))
                for hq in range(2):
                    q0 = hq * HT
                    ps = pmisc.tile([128, 512], f32, tag="pmisc")
                    for fc in range(FC):
                        nc.tensor.matmul(ps[:, 0:HT], w2mc[:, fc, :],
                                         ffT[:, fc, ds(q0, HT)],
                                         start=(fc == 0), stop=(fc == FC - 1))
                    nc.vector.scalar_tensor_tensor(
                        xT[:, mc, ds(q0, HT)], ps[:, 0:HT],
                        b2_sb[:, mc:mc + 1], xT[:, mc, ds(q0, HT)],
                        OP.add, OP.add)
            # ---- LN2 ----------------------------------------------------
            for hq in range(2):
                layernorm_half(g2_sb, be2_sb, hq, xbf_cur, late=True)

        # ---- output ----------------------------------------------------
        nc.sync.dma_start(out=outT[:, :].rearrange("(c p) t -> p c t", p=128), in_=xT)
        ctx.close()

    nc.compile()
    return nc


def _prepare_host(inputs):
    src = np.asarray(inputs["src"]).astype(np.int64)
    emb = np.asarray(inputs["emb"], dtype=np.float32)
    x = emb[src]                                    # [B, S, D] f32
    pos = np.arange(B, dtype=np.float32)[:, None]
    div = np.exp(np.arange(0, D, 2, dtype=np.float32) * (-np.log(10000.0) / D))
    pe = np.zeros((B, D), np.float32)
    pe[:, 0::2] = np.sin(pos / div)
    pe[:, 1::2] = np.cos(pos / div)
    x = x + pe[:, None, :]

    mask = np.asarray(inputs["src_mask"]).reshape(B, S)
    mbias = np.where(mask != 0, 0.0, -1e9).astype(np.float32)   # [B, S]

    f = np.float32
    shared = {
        "wq": np.ascontiguousarray(np.asarray(inputs["Wq"], f).astype(BF16)),
        "wk": np.ascontiguousarray(np.asarray(inputs["Wk"], f).astype(BF16)),
        "wv": np.ascontiguousarray(np.asarray(inputs["Wv"], f).astype(BF16)),
        "wo": np.ascontiguousarray(np.asarray(inputs["Wo"], f).astype(BF16)),
        "w1": np.ascontiguousarray(np.asarray(inputs["W1"], f).astype(BF16)),
        "w2p": np.ascontiguousarray(
            np.asarray(inputs["W2"], f).reshape(L, FC, 128, DC, 128)
            .transpose(0, 3, 1, 2, 4).astype(BF16)),
        "bq": np.ascontiguousarray(np.asarray(inputs["bq"], f)),
        "bk": np.ascontiguousarray(np.asarray(inputs["bk"], f)),
        "bvb": np.ascontiguousarray(np.asarray(inputs["bv"], f).astype(BF16)),
        "bo": np.ascontiguousarray(np.asarray(inputs["bo"], f)),
        "b1": np.ascontiguousarray(np.asarray(inputs["b1"], f)),
        "b2": np.ascontiguousarray(np.asarray(inputs["b2"], f)),
        "g1": np.ascontiguousarray(np.asarray(inputs["g1"], f)),
        "be1": np.ascontiguousarray(np.asarray(inputs["be1"], f)),
        "g2": np.ascontiguousarray(np.asarray(inputs["g2"], f)),
        "be2": np.ascontiguousarray(np.asarray(inputs["be2"], f)),
    }
    in_maps = []
    for i in range(NCORES):
        b = i // R
        t0 = (i % R) * T
        m = dict(shared)
        m["x0T"] = np.ascontiguousarray(x[b, t0:t0 + T, :].T.astype(np.float32))
        m["maskb"] = np.ascontiguousarray(
            mbias[b, t0:t0 + T].reshape(TC, 128).T)
        in_maps.append(m)
    return in_maps


def _run(in_maps, trace=False):
    from concourse.bass_utils import run_bass_kernel_spmd
    if "nc" not in _CACHE:
        _CACHE["nc"] = _build_nc()
    nc = _CACHE["nc"]
    res = run_bass_kernel_spmd(nc, in_maps, core_ids=list(range(NCORES)),
                               trace=trace)
    outs = res.results
    y = np.zeros((B, S, D), np.float32)
    for i in range(NCORES):
        b = i // R
        t0 = (i % R) * T
        y[b, t0:t0 + T, :] = outs[i]["outT"].T
    return y, res


def kernel(**inputs) -> np.ndarray:
    in_maps = _prepare_host(inputs)
    y, _ = _run(in_maps, trace=False)
    return y


def kernel_traced(**inputs):
    """Same as kernel() but returns (output, BassKernelResults with profile)."""
    in_maps = _prepare_host(inputs)
    return _run(in_maps, trace=True)
